# revision 1
# baseline (speedup 1.0000x reference)
"""Trainium2 Bass kernel: batch row-sharded grouped GEMM (MoE routing).

Contract: kernel(x, weight, num_inputs_per_group) takes FULL inputs
  x (32768, 2048) f32, weight (16, 2048, 2048) f32, num_inputs_per_group (16,) i32
and returns the FULL output (32768, 2048) f32, where token row i is multiplied
by weight[seg[i]] with seg = repeat(arange(16), num, total_repeat_length=32768)
(contiguous groups).

Distribution strategy (no collectives needed): tokens are split into contiguous
128-row blocks; each of the 8 cores gets an equal number of blocks plus the
weights for the experts its blocks use (expert/token parallelism — sanctioned
by the sharding hint since E=16 >= 8). Each core computes a dense grouped GEMM
locally and the host concatenates the per-core outputs.

Device kernel: fp32r matmuls (TF32-like input rounding, fp32 accumulation,
full PE rate). Host pre-lays-out both operands so every DMA moves multi-KB
contiguous chunks per partition:
  x  -> per-group tiles [128(d%128), 16(d//128), 256(token)]  (16 KB chunks)
  w  -> per-(slot, n-set) tiles [128, 16, 512]                (32 KB chunks)
Weight n-sets give n-granular dependencies: the first 4 MiB set unlocks every
token block's n=0 PSUM group ~12us after launch instead of the full 16 MiB
expert load gating the first block.
"""

import sys

sys.path.insert(0, "/opt/trn_rl_repo")

import numpy as np

import concourse.bacc as bacc
import concourse.mybir as mybir
from concourse.bass_utils import run_bass_kernel_spmd
from concourse.tile import TileContext
from concourse.tile_rust import add_dep_helper

N_TOK, D_IN, D_OUT, N_EXP = 32768, 2048, 2048, 16
NCORES = 8
PB = 128  # token block = PSUM partition count
NT = 512  # matmul moving free dim = one fp32 PSUM bank
KT = D_IN // PB  # 16 k-tiles
NTILES = D_OUT // NT  # 4 output column sets
MG_BLOCKS = 2  # token blocks per x group tile
MGT = MG_BLOCKS * PB  # tokens per group tile

# Introspection hooks for test.py (harness just calls kernel()).
TRACE = False
LAST_RESULTS = None


def _seg_from_groups(num):
    """Replicate jnp.repeat(arange(E), num, total_repeat_length=N) semantics."""
    num = np.asarray(num, dtype=np.int64)
    reps = np.repeat(np.arange(N_EXP, dtype=np.int32), np.maximum(num, 0))
    if len(reps) >= N_TOK:
        return reps[:N_TOK]
    pad = reps[-1] if len(reps) else np.int32(0)
    return np.concatenate([reps, np.full(N_TOK - len(reps), pad, np.int32)])


def _run_groups(runs):
    """Split each run's blocks into m-groups of up to MG_BLOCKS blocks."""
    groups = []  # (run_idx, g_blocks)
    for ri, (_, nb) in enumerate(runs):
        b = 0
        while b < nb:
            g = min(MG_BLOCKS, nb - b)
            groups.append((ri, g))
            b += g
    return groups


def _build_nc(n_blocks_core, runs, n_slots):
    """Build the per-core SPMD kernel.

    runs: list of (slot, n_blocks) with sum(n_blocks) == n_blocks_core.
    Every core runs this same program; per-core data (x slice, slot->expert
    weight choice) lives in the input maps.
    """
    T_core = n_blocks_core * PB
    f32 = mybir.dt.float32
    f32r = mybir.dt.float32r
    groups = _run_groups(runs)

    nc = bacc.Bacc("TRN2", target_bir_lowering=False, debug=False, num_devices=NCORES)
    xh = nc.dram_tensor("xh", [len(groups), PB, KT, MGT], f32r, kind="ExternalInput")
    w = nc.dram_tensor("w", [n_slots, NTILES, PB, KT, NT], f32r, kind="ExternalInput")
    out = nc.dram_tensor("out", [T_core, D_OUT], f32, kind="ExternalOutput")

    with TileContext(nc) as tc:
        with (
            tc.tile_pool(name="wpool", bufs=4) as wpool,
            tc.tile_pool(name="xpool", bufs=4) as xpool,
            tc.tile_pool(name="opool", bufs=2) as opool,
            tc.tile_pool(name="pspool", bufs=7, space="PSUM") as pspool,
            tc.tile_pool(name="warmpool", bufs=1, space="PSUM") as warmpool,
        ):
            # Warm-up: ~30 throwaway matmuls keep the PE busy (and its HAM
            # clock gate at 8/8) through the ~20us HBM-bound ramp while the
            # first weight set and x tiles stream in. fp32 (not fp32r) so the
            # memset-produced scratch needs no fp32r rounding producer.
            wsrc = xpool.tile([PB, PB], f32, name="warm_src", tag="warm")
            nc.vector.memset(wsrc, 0.0)
            wps = warmpool.tile([PB, PB], f32, name="warm_ps", tag="warm_ps")
            for _ in range(30):
                nc.tensor.matmul(wps, wsrc, wsrc, start=True, stop=True)
            run_group0 = []
            g0 = 0
            for ri in range(len(runs)):
                run_group0.append(g0)
                g0 += sum(1 for r, _ in groups if r == ri)

            blk = 0
            for ri, (slot, nb) in enumerate(runs):
                # Two passes over this run's tokens: pass 0 consumes weight
                # n-sets {0,1}, pass 1 consumes {2,3}. The PE executes in
                # program order, so each 4 MiB n-set only gates work that
                # genuinely needs it, and each set has a half-run (~100us) of
                # compute as prefetch slack — x is re-streamed per pass to
                # keep SBUF small (DMA has the headroom; PE is the
                # bottleneck). Weight n-sets go on the SP HWDGE ring; x/out
                # use the ACT ring so they never queue behind a weight
                # stream. Each n-set is split into two k-half DMAs so the
                # first PSUM group can start after ~2 MiB. The s2/s3 DMAs are
                # artificially made dependent on early pass-0 matmuls: HBM
                # bandwidth is the ramp bottleneck, and without the dep they
                # stream immediately and starve the x/s0/s1 loads the ramp
                # actually needs.
                wt = []
                w_dmas = []
                for n in range(NTILES):
                    t = wpool.tile(
                        [PB, KT, NT], f32r, name=f"w_s{slot}_n{n}", tag="w"
                    )
                    kh = KT // 2
                    d1 = nc.sync.dma_start(out=t[:, :kh, :], in_=w[slot, n, :, :kh, :])
                    d2 = nc.sync.dma_start(out=t[:, kh:, :], in_=w[slot, n, :, kh:, :])
                    wt.append(t)
                    w_dmas.append((d1, d2))
                pass_sets = [[0, 1], [2, 3]]
                chunk_first_mm = {}  # chunk ordinal in pass 0 -> first MM inst
                deferred = []
                for p, nset in enumerate(pass_sets):
                    gi = run_group0[ri]
                    chunk = 0
                    b = 0
                    while b < nb:
                        _, g = groups[gi]
                        xt = xpool.tile(
                            [PB, KT, MGT], f32r, name=f"xt_{gi}_{p}", tag="xt"
                        )
                        nc.scalar.dma_start(out=xt, in_=xh[gi])
                        ots = []
                        for mb in range(g):
                            ot = opool.tile(
                                [PB, len(nset) * NT],
                                f32,
                                name=f"o_{blk + b + mb}_{p}",
                                tag="o",
                            )
                            ots.append(ot)
                        # (n, mb, k) order: each PSUM group's 16 matmuls
                        # accumulate over k, and the second weight n-set of a
                        # pass isn't touched until ~10us of compute into it.
                        def emit_groups(xt_, ots_, g_, js, base_b, first_anchor):
                            for j in js:
                                n = nset[j]
                                for mb in range(g_):
                                    ps = pspool.tile(
                                        [PB, NT], f32, name="ps", tag="ps"
                                    )
                                    for k in range(KT):
                                        mm = nc.tensor.matmul(
                                            ps,
                                            xt_[:, k, mb * PB : (mb + 1) * PB],
                                            wt[n][:, k, :],
                                            start=(k == 0),
                                            stop=(k == KT - 1),
                                        )
                                        if first_anchor and j == js[0] and mb == 0 and k == 0:
                                            chunk_first_mm[first_anchor[0]] = mm
                                    nc.vector.tensor_copy(
                                        out=ots_[mb][:, j * NT : (j + 1) * NT],
                                        in_=ps,
                                    )

                        def emit_outs(ots_, g_, base_b):
                            for mb in range(g_):
                                row = (blk + base_b + mb) * PB
                                nc.scalar.dma_start(
                                    out=out[
                                        row : row + PB,
                                        nset[0] * NT : (nset[-1] + 1) * NT,
                                    ],
                                    in_=ots_[mb],
                                )

                        all_js = list(range(len(nset)))
                        if (
                            ri == 0
                            and p == 0
                            and chunk == 0
                            and len(nset) > 1
                            and len(groups) > 1
                            and groups[1][0] == 0
                        ):
                            # Ramp: run chunk 0's n0 groups now and defer its
                            # n1 groups until after chunk 1's n0 groups (the
                            # x tile stays resident), doubling the compute
                            # available to hide the second weight set's
                            # HBM-bound arrival.
                            emit_groups(xt, ots, g, all_js[:1], b, (chunk,))
                            deferred.append((xt, ots, g, b))
                        else:
                            emit_groups(
                                xt, ots, g, all_js, b, (chunk,) if p == 0 else None
                            )
                            emit_outs(ots, g, b)
                            for xt_, ots_, g_, b_ in deferred:
                                emit_groups(xt_, ots_, g_, all_js[1:], b_, None)
                                emit_outs(ots_, g_, b_)
                            deferred = []
                        gi += 1
                        chunk += 1
                        b += g
                # Hold back this run's s2/s3 streams until its pass-0 compute
                # is underway (see comment above).
                nchunks = chunk
                for n, anchor in ((2, 1), (3, 2)):
                    a = chunk_first_mm.get(min(anchor, nchunks - 1))
                    if a is not None and nchunks > 2:
                        for dd in w_dmas[n]:
                            add_dep_helper(
                                dd.ins,
                                a.ins,
                                sync=True,
                                reason="stagger weight n-set stream behind ramp",
                            )
                blk += nb
    nc.compile()
    return nc


def _host_layout_x(x_core, runs):
    """Pack a core's tokens [T, D] into group tiles [NG, 128, 16, 256]."""
    groups = _run_groups(runs)
    xh = np.zeros((len(groups), PB, KT, MGT), dtype=np.float32)
    t0 = 0
    for i, (_, g) in enumerate(groups):
        gt = g * PB
        blockT = x_core[t0 : t0 + gt]  # [gt, D]
        # (t, k, p) -> (p, k, t)
        xh[i, :, :, :gt] = blockT.reshape(gt, KT, PB).transpose(2, 1, 0)
        t0 += gt
    return np.ascontiguousarray(xh)


def _host_layout_w(w_slots):
    """Pack slot weights [S, D, O] into n-set tiles [S, 4, 128, 16, 512]."""
    S = w_slots.shape[0]
    # (s, k, p, n, j) -> (s, n, p, k, j)
    return np.ascontiguousarray(
        w_slots.reshape(S, KT, PB, NTILES, NT).transpose(0, 3, 2, 1, 4)
    )


def kernel(x, weight, num_inputs_per_group):
    global LAST_RESULTS
    x = np.ascontiguousarray(np.asarray(x, dtype=np.float32))
    weight = np.ascontiguousarray(np.asarray(weight, dtype=np.float32))
    seg = _seg_from_groups(num_inputs_per_group)

    # --- plan: map 128-token blocks to experts ---------------------------------
    aligned = all(
        np.all(seg[i * PB : (i + 1) * PB] == seg[i * PB]) for i in range(N_TOK // PB)
    )
    if aligned:
        block_expert = seg[::PB].astype(np.int64)  # (256,)
        block_tokens = None  # identity: block b covers rows [b*128, (b+1)*128)
    else:
        # Generic fallback: pad each contiguous expert segment to a 128 multiple
        # via a host-side gather; output rows are scattered back afterwards.
        bounds = np.flatnonzero(np.diff(seg)) + 1
        starts = np.concatenate([[0], bounds])
        ends = np.concatenate([bounds, [N_TOK]])
        blocks, experts = [], []
        for s, e in zip(starts, ends):
            idx = np.arange(s, e, dtype=np.int64)
            padded = -np.ones(int(np.ceil(len(idx) / PB)) * PB, dtype=np.int64)
            padded[: len(idx)] = idx
            for b0 in range(0, len(padded), PB):
                blocks.append(padded[b0 : b0 + PB])
                experts.append(int(seg[s]))
        while len(blocks) % NCORES:
            blocks.append(-np.ones(PB, dtype=np.int64))
            experts.append(0)
        block_tokens = np.stack(blocks)  # (n_blocks, 128) token ids, -1 = pad
        block_expert = np.asarray(experts, dtype=np.int64)

    n_blocks = len(block_expert)
    n_blocks_core = n_blocks // NCORES
    per_core_experts = block_expert.reshape(NCORES, n_blocks_core)

    # Run-length encode each core's block->expert map; if all cores share the
    # same run-length pattern we can use compact per-run weight slots.
    def rle(v):
        runs = []
        for e in v:
            if runs and runs[-1][0] == e:
                runs[-1][1] += 1
            else:
                runs.append([int(e), 1])
        return runs

    core_runs = [rle(per_core_experts[c]) for c in range(NCORES)]
    lengths0 = [n for _, n in core_runs[0]]
    if all([n for _, n in core_runs[c]] == lengths0 for c in range(NCORES)):
        runs = [(s, n) for s, (_, n) in enumerate(core_runs[0])]
        slot_experts = [[e for e, _ in core_runs[c]] for c in range(NCORES)]
    else:
        runs = [(b, 1) for b in range(n_blocks_core)]
        slot_experts = [list(per_core_experts[c]) for c in range(NCORES)]
    n_slots = len(runs)

    # --- per-core inputs -------------------------------------------------------
    in_maps = []
    for c in range(NCORES):
        if block_tokens is None:
            rows = slice(c * n_blocks_core * PB, (c + 1) * n_blocks_core * PB)
            xc = x[rows]
        else:
            tok = block_tokens[c * n_blocks_core : (c + 1) * n_blocks_core].ravel()
            xc = np.where(tok[:, None] >= 0, x[np.maximum(tok, 0)], 0.0).astype(
                np.float32
            )
        in_maps.append(
            {
                "xh": _host_layout_x(xc, runs),
                "w": _host_layout_w(weight[slot_experts[c]]),
            }
        )

    nc = _build_nc(n_blocks_core, runs, n_slots)
    res = run_bass_kernel_spmd(nc, in_maps, core_ids=list(range(NCORES)), trace=TRACE)
    LAST_RESULTS = res

    # --- unshard ---------------------------------------------------------------
    outs = [res.results[c]["out"] for c in range(NCORES)]
    if block_tokens is None:
        return np.concatenate(outs, axis=0)
    full = np.zeros((N_TOK, D_OUT), dtype=np.float32)
    flat_tok = block_tokens.ravel()
    flat_out = np.concatenate(outs, axis=0)
    valid = flat_tok >= 0
    full[flat_tok[valid]] = flat_out[valid]
    return full



# revision 4
# speedup vs baseline: 1.1293x; 1.1293x over previous
"""Trainium2 Bass kernel: batch row-sharded grouped GEMM (MoE routing).

Contract: kernel(x, weight, num_inputs_per_group) takes FULL inputs
  x (32768, 2048) f32, weight (16, 2048, 2048) f32, num_inputs_per_group (16,) i32
and returns the FULL output (32768, 2048) f32, where token row i is multiplied
by weight[seg[i]] with seg = repeat(arange(16), num, total_repeat_length=32768)
(contiguous groups).

Distribution strategy (no collectives needed): tokens are split into contiguous
128-row blocks; each of the 8 cores gets an equal number of blocks plus the
weights for the experts its blocks use (expert/token parallelism — sanctioned
by the sharding hint since E=16 >= 8). Each core computes a dense grouped GEMM
locally and the host concatenates the per-core outputs.

Device kernel (bf16 inputs, fp32 PSUM accumulation): the host rounds x and w
to bf16 (rel err ~1.6e-3, far under the 2e-2 gate), which halves HBM traffic
vs fp32r and lets a core keep a whole expert's weights (8 MiB) resident in
SBUF. Per expert run the kernel makes 4 n-major sweeps (one per 512-col
output set) over the run's token blocks with the x tiles resident, so x is
streamed exactly once and each 2 MiB weight n-set has a whole sweep (~55us)
of prefetch slack — the PE only ever waits for the first ~3 MiB at launch.
Weights ride the Sync-engine DMA ring; x and outputs ride the Scalar ring
with x tiles front-loaded per sub-run so output writes can never delay them.
"""

import sys

sys.path.insert(0, "/opt/trn_rl_repo")

import numpy as np

try:
    import ml_dtypes

    BF16 = np.dtype(ml_dtypes.bfloat16)
except Exception:  # pragma: no cover
    BF16 = None

import concourse.bacc as bacc
import concourse.mybir as mybir
from concourse.bass_utils import run_bass_kernel_spmd
from concourse.tile import TileContext

N_TOK, D_IN, D_OUT, N_EXP = 32768, 2048, 2048, 16
NCORES = 8
PB = 128  # token block = PSUM partition count
NT = 512  # matmul moving free dim = one fp32 PSUM bank
KT = D_IN // PB  # 16 k-tiles
NTILES = D_OUT // NT  # 4 output column sets
MG_BLOCKS = 2  # token blocks per x group tile
MGT = MG_BLOCKS * PB  # tokens per group tile
SUBRUN = 8  # max x group tiles held resident per n-major sweep set

# Introspection hooks for test.py (harness just calls kernel()).
TRACE = False
LAST_RESULTS = None


def _seg_from_groups(num):
    """Replicate jnp.repeat(arange(E), num, total_repeat_length=N) semantics."""
    num = np.asarray(num, dtype=np.int64)
    reps = np.repeat(np.arange(N_EXP, dtype=np.int32), np.maximum(num, 0))
    if len(reps) >= N_TOK:
        return reps[:N_TOK]
    pad = reps[-1] if len(reps) else np.int32(0)
    return np.concatenate([reps, np.full(N_TOK - len(reps), pad, np.int32)])


def _run_groups(runs):
    """Split each run's blocks into m-groups of up to MG_BLOCKS blocks."""
    groups = []  # (run_idx, g_blocks)
    for ri, (_, nb) in enumerate(runs):
        b = 0
        while b < nb:
            g = min(MG_BLOCKS, nb - b)
            groups.append((ri, g))
            b += g
    return groups


def _build_nc(n_blocks_core, runs, n_slots):
    """Build the per-core SPMD kernel.

    runs: list of (slot, n_blocks) with sum(n_blocks) == n_blocks_core.
    Every core runs this same program; per-core data (x slice, slot->expert
    weight choice) lives in the input maps.
    """
    T_core = n_blocks_core * PB
    f32 = mybir.dt.float32
    bf16 = mybir.dt.bfloat16
    groups = _run_groups(runs)
    # first block index of each group (global within the core)
    block_of = []
    b = 0
    for _, g in groups:
        block_of.append(b)
        b += g
    run_groups = [[] for _ in runs]  # per run: list of (global gi, g)
    for gi, (ri, g) in enumerate(groups):
        run_groups[ri].append((gi, g))

    nc = bacc.Bacc("TRN2", target_bir_lowering=False, debug=False, num_devices=NCORES)
    xh = nc.dram_tensor("xh", [len(groups), PB, KT, MGT], bf16, kind="ExternalInput")
    w = nc.dram_tensor("w", [n_slots, NTILES, PB, KT, NT], bf16, kind="ExternalInput")
    out = nc.dram_tensor("out", [T_core, D_OUT], f32, kind="ExternalOutput")

    with TileContext(nc) as tc:
        with (
            tc.tile_pool(name="wpool", bufs=5) as wpool,
            tc.tile_pool(name="xpool", bufs=SUBRUN + 2) as xpool,
            tc.tile_pool(name="opool", bufs=12) as opool,
            tc.tile_pool(name="warmsrc", bufs=1) as warmsrc,
            tc.tile_pool(name="pspool", bufs=7, space="PSUM") as pspool,
            tc.tile_pool(name="warmpool", bufs=1, space="PSUM") as warmpool,
        ):
            # Warm-up: ~30 throwaway matmuls keep the PE busy (and its HAM
            # clock gate at 8/8) through the input-ready barrier + first-DMA
            # window. fp32 (not fp32r) so the memset-produced scratch needs
            # no fp32r rounding producer.
            wsrc = warmsrc.tile([PB, PB], f32, name="warm_src", tag="warm")
            nc.vector.memset(wsrc, 0.0)
            wps = warmpool.tile([PB, PB], f32, name="warm_ps", tag="warm_ps")
            for _ in range(22):
                nc.tensor.matmul(wps, wsrc, wsrc, start=True, stop=True)

            xt = {}  # global gi -> live tile

            def emit_x_dma(gi, split):
                t = xpool.tile([PB, KT, MGT], bf16, name=f"xt_{gi}", tag="xt")
                kh = KT // split
                for q in range(split):
                    nc.scalar.dma_start(
                        out=t[:, q * kh : (q + 1) * kh, :],
                        in_=xh[gi, :, q * kh : (q + 1) * kh, :],
                    )
                xt[gi] = t

            for ri, (slot, nb) in enumerate(runs):
                if ri == 0:
                    # Launch-critical pieces ride the Sync ring, which the
                    # runtime arms ~4us before the Scalar ring: the first x
                    # tile's k-lower half goes ahead of the weights, the
                    # first weight n-set is split into k-eighths so each
                    # ~0.25 MiB arrival unlocks more of the interleaved
                    # opening chains, and the x upper half leads the Scalar
                    # ring.
                    t = xpool.tile([PB, KT, MGT], bf16, name="xt_0", tag="xt")
                    nc.sync.dma_start(
                        out=t[:, : KT // 2, :], in_=xh[0, :, : KT // 2, :]
                    )
                    nc.scalar.dma_start(
                        out=t[:, KT // 2 :, :], in_=xh[0, :, KT // 2 :, :]
                    )
                    xt[0] = t
                # This run's weights: 4 n-set tiles, resident for the whole
                # run; each n-set has a whole sweep of prefetch slack.
                wt = []
                for n in range(NTILES):
                    t = wpool.tile([PB, KT, NT], bf16, name=f"w_r{ri}_n{n}", tag="w")
                    split = 8 if (ri == 0 and n == 0) else 2
                    kh = KT // split
                    for q in range(split):
                        nc.sync.dma_start(
                            out=t[:, q * kh : (q + 1) * kh, :],
                            in_=w[slot, n, :, q * kh : (q + 1) * kh, :],
                        )
                    wt.append(t)

                rg = run_groups[ri]
                for s0 in range(0, len(rg), SUBRUN):
                    sub = rg[s0 : s0 + SUBRUN]
                    last_sub = s0 + SUBRUN >= len(rg)
                    for gi, g in sub:
                        if gi not in xt:
                            emit_x_dma(gi, split=2 if gi == 0 else 1)
                    if ri == 0 and s0 == 0:
                        # Opening chains for the first x tile, interleaved
                        # k-pair by k-pair across the tile's blocks so each
                        # arriving 0.25 MiB weight piece feeds g blocks'
                        # worth of matmuls instead of one chain's.
                        g0_g = sub[0][1]
                        pss = [
                            pspool.tile([PB, NT], f32, name="ps", tag="ps")
                            for _ in range(g0_g)
                        ]
                        for kp in range(0, KT, 2):
                            for mb in range(g0_g):
                                for k in (kp, kp + 1):
                                    nc.tensor.matmul(
                                        pss[mb],
                                        xt[0][:, k, mb * PB : (mb + 1) * PB],
                                        wt[0][:, k, :],
                                        start=(k == 0),
                                        stop=(k == KT - 1),
                                    )
                        for mb in range(g0_g):
                            ot = opool.tile([PB, NT], f32, name="o", tag="o")
                            nc.vector.tensor_copy(out=ot, in_=pss[mb])
                            row = (block_of[sub[0][0]] + mb) * PB
                            nc.scalar.dma_start(
                                out=out[row : row + PB, 0:NT], in_=ot
                            )
                    for n in range(NTILES):
                        # Before the final sweep of this run's last sub-run,
                        # prefetch the next run's first two x tiles so the
                        # run boundary never waits on the x stream.
                        if n == NTILES - 1 and last_sub and ri + 1 < len(runs):
                            for gi, _ in run_groups[ri + 1][:2]:
                                emit_x_dma(gi, split=1)
                        for sj, (gi, g) in enumerate(sub):
                            if ri == 0 and s0 == 0 and n == 0 and sj == 0:
                                continue  # opening chains already emitted
                            for mb in range(g):
                                ps = pspool.tile([PB, NT], f32, name="ps", tag="ps")
                                for k in range(KT):
                                    nc.tensor.matmul(
                                        ps,
                                        xt[gi][:, k, mb * PB : (mb + 1) * PB],
                                        wt[n][:, k, :],
                                        start=(k == 0),
                                        stop=(k == KT - 1),
                                    )
                                ot = opool.tile([PB, NT], f32, name="o", tag="o")
                                nc.vector.tensor_copy(out=ot, in_=ps)
                                row = (block_of[gi] + mb) * PB
                                nc.scalar.dma_start(
                                    out=out[row : row + PB, n * NT : (n + 1) * NT],
                                    in_=ot,
                                )
                    # Tiles of a finished sub-run are recycled by the pool.
                    for gi, g in sub:
                        del xt[gi]
    nc.compile()
    return nc


def _host_layout_x(x_core, runs):
    """Pack a core's bf16 tokens [T, D] into group tiles [NG, 128, 16, 256]."""
    groups = _run_groups(runs)
    xh = np.zeros((len(groups), PB, KT, MGT), dtype=BF16)
    t0 = 0
    for i, (_, g) in enumerate(groups):
        gt = g * PB
        blockT = x_core[t0 : t0 + gt]  # [gt, D]
        # (t, k, p) -> (p, k, t)
        xh[i, :, :, :gt] = blockT.reshape(gt, KT, PB).transpose(2, 1, 0)
        t0 += gt
    return np.ascontiguousarray(xh)


def _host_layout_w(w_slots):
    """Pack bf16 slot weights [S, D, O] into n-set tiles [S, 4, 128, 16, 512]."""
    S = w_slots.shape[0]
    # (s, k, p, n, j) -> (s, n, p, k, j)
    return np.ascontiguousarray(
        w_slots.reshape(S, KT, PB, NTILES, NT).transpose(0, 3, 2, 1, 4)
    )


def kernel(x, weight, num_inputs_per_group):
    global LAST_RESULTS
    x = np.asarray(x, dtype=np.float32)
    weight = np.asarray(weight, dtype=np.float32)
    seg = _seg_from_groups(num_inputs_per_group)
    x_bf = np.ascontiguousarray(x.astype(BF16))
    w_bf = np.ascontiguousarray(weight.astype(BF16))

    # --- plan: map 128-token blocks to experts ---------------------------------
    aligned = all(
        np.all(seg[i * PB : (i + 1) * PB] == seg[i * PB]) for i in range(N_TOK // PB)
    )
    if aligned:
        block_expert = seg[::PB].astype(np.int64)  # (256,)
        block_tokens = None  # identity: block b covers rows [b*128, (b+1)*128)
    else:
        # Generic fallback: pad each contiguous expert segment to a 128 multiple
        # via a host-side gather; output rows are scattered back afterwards.
        bounds = np.flatnonzero(np.diff(seg)) + 1
        starts = np.concatenate([[0], bounds])
        ends = np.concatenate([bounds, [N_TOK]])
        blocks, experts = [], []
        for s, e in zip(starts, ends):
            idx = np.arange(s, e, dtype=np.int64)
            padded = -np.ones(int(np.ceil(len(idx) / PB)) * PB, dtype=np.int64)
            padded[: len(idx)] = idx
            for b0 in range(0, len(padded), PB):
                blocks.append(padded[b0 : b0 + PB])
                experts.append(int(seg[s]))
        while len(blocks) % NCORES:
            blocks.append(-np.ones(PB, dtype=np.int64))
            experts.append(0)
        block_tokens = np.stack(blocks)  # (n_blocks, 128) token ids, -1 = pad
        block_expert = np.asarray(experts, dtype=np.int64)

    n_blocks = len(block_expert)
    n_blocks_core = n_blocks // NCORES
    per_core_experts = block_expert.reshape(NCORES, n_blocks_core)

    # Run-length encode each core's block->expert map; if all cores share the
    # same run-length pattern we can use compact per-run weight slots.
    def rle(v):
        runs = []
        for e in v:
            if runs and runs[-1][0] == e:
                runs[-1][1] += 1
            else:
                runs.append([int(e), 1])
        return runs

    core_runs = [rle(per_core_experts[c]) for c in range(NCORES)]
    lengths0 = [n for _, n in core_runs[0]]
    if all([n for _, n in core_runs[c]] == lengths0 for c in range(NCORES)):
        runs = [(s, n) for s, (_, n) in enumerate(core_runs[0])]
        slot_experts = [[e for e, _ in core_runs[c]] for c in range(NCORES)]
    else:
        runs = [(b, 1) for b in range(n_blocks_core)]
        slot_experts = [list(per_core_experts[c]) for c in range(NCORES)]
    n_slots = len(runs)

    # --- per-core inputs -------------------------------------------------------
    in_maps = []
    for c in range(NCORES):
        if block_tokens is None:
            rows = slice(c * n_blocks_core * PB, (c + 1) * n_blocks_core * PB)
            xc = x_bf[rows]
        else:
            tok = block_tokens[c * n_blocks_core : (c + 1) * n_blocks_core].ravel()
            xc = np.where(
                tok[:, None] >= 0, x_bf[np.maximum(tok, 0)], np.zeros((), BF16)
            ).astype(BF16)
        in_maps.append(
            {
                "xh": _host_layout_x(xc, runs),
                "w": _host_layout_w(w_bf[slot_experts[c]]),
            }
        )

    nc = _build_nc(n_blocks_core, runs, n_slots)
    res = run_bass_kernel_spmd(nc, in_maps, core_ids=list(range(NCORES)), trace=TRACE)
    LAST_RESULTS = res

    # --- unshard ---------------------------------------------------------------
    outs = [res.results[c]["out"] for c in range(NCORES)]
    if block_tokens is None:
        return np.concatenate(outs, axis=0)
    full = np.zeros((N_TOK, D_OUT), dtype=np.float32)
    flat_tok = block_tokens.ravel()
    flat_out = np.concatenate(outs, axis=0)
    valid = flat_tok >= 0
    full[flat_tok[valid]] = flat_out[valid]
    return full
